# revision 4
# baseline (speedup 1.0000x reference)
"""Bidirectional 2-layer GRU encoder on 8 Trainium2 NeuronCores.

B=64, S=512, IN=512, H=1024. The reference's "backward" direction is a
feature-axis flip of x_t (not time reversal), so all 4 GRU cells scan forward
in time. Sharding: 8 cores = 2 cells (f/b chain) x 4 batch quarters (16 rows
each); each core runs its chain completely locally (no cross-core comm):

  phase A  : gi0 = Xq @ w_ih0^T + bias           (big GEMM, per core)
  scan 0   : layer-0 GRU scan, 512 steps          (emits transposed h states)
  phase C  : gi1 = H0q @ w_ih1^T + bias           (big GEMM from scan-0 states)
  scan 1   : layer-1 GRU scan, 512 steps          (emits h1 sequence)

Recurrent matmuls run with the batch-transposed state as the PE stationary
operand and w_hh^T streaming (fp32r: 1 cycle/row at N=512). Biases are
pre-folded into gi (b_ih for all gates + b_hh for r,z); b_hh_n is added on
the h-side inside the scan, matching PyTorch GRU gate math exactly.
"""

import os
import sys
import numpy as np

import concourse.bass as bass
import concourse.mybir as mybir
from concourse.tile import TileContext
from concourse.vector_clock import ScopedClock

B, S, IN, H = 64, 512, 512, 1024
G = 3 * H            # 3072 gate columns, order [r | z | n]
NCORES = 8
BQ = B // 4          # 16 batch rows per core
TCHUNK = 128         # scan steps per launch
AF = mybir.ActivationFunctionType
DT = mybir.dt

# ----------------------------------------------------------------- walrus fixes


def _patched_drain_and_barrier(self, tick_clock, wait_clock):
    nc = self.nc
    probe = nc.sync.nop(nofuse=True)
    wait_clock.add_sem_waits(probe.ins, ScopedClock({None: tick_clock.global_clock}))
    si = probe.ins.sync_info
    waits = list(si.on_wait) if si is not None else []
    probe.ins.sync_info = mybir.SyncInfo(on_wait=waits[:1], on_update=[])
    for w in waits[1:]:
        n2 = nc.sync.nop(nofuse=True)
        n2.ins.sync_info = mybir.SyncInfo(on_wait=[w], on_update=[])
    nc.sync.drain()
    nc.all_engine_barrier()
    popped = nc._tile_sem_poison_stack.pop()
    assert popped is self._sem_poison
    nc.clear_and_free_semaphores(list(self.sems.allocated().values()))
    nc.all_engine_barrier()


TileContext._drain_and_barrier = _patched_drain_and_barrier


def _split_multiwaits(nc):
    """This container's walrus accepts at most one sync-wait per instruction;
    hoist extras onto same-engine NOPs (sequencers are strict FIFO)."""
    for f in nc.m.functions:
        for bb in f.blocks:
            insts = bb.instructions
            i = 0
            while i < len(insts):
                inst = insts[i]
                si = inst.sync_info
                if si is not None and len(si.on_wait) > 1:
                    waits = list(si.on_wait)
                    for j, w in enumerate(waits[:-1]):
                        nop = mybir.InstNoOp(
                            name=f"{inst.name}.wsplit{j}", engine=inst.engine,
                            sync_info=mybir.SyncInfo(on_wait=[w], on_update=[]),
                            bass_nofuse=True)
                        insts.insert(i, nop)
                        i += 1
                    inst.sync_info = mybir.SyncInfo(
                        on_wait=[waits[-1]], on_update=list(si.on_update))
                i += 1


# ----------------------------------------------------------------- kernel builders


def build_gemm(k_chunks, from_ht):
    """gi[MR, G] = XT^T @ W + bias.  XT is the pre-transposed left operand
    [128*k_chunks, MR] (from_ht=False, one DRAM tensor) or four scan-chunk
    state dumps [TCHUNK, 128, 8, 16] (from_ht=True). W is [128*k_chunks, G].
    bias is replicated [128, G]. Row m of gi is (t, b) = (m // 16, m % 16)."""
    MR = S * BQ  # 8192
    nc = bass.Bass(trn_type="TRN2", num_devices=NCORES)
    if from_ht:
        xts = [nc.dram_tensor(f"xt{i}", [TCHUNK, 128, 8, BQ], DT.float32r,
                              kind="ExternalInput") for i in range(S // TCHUNK)]
    else:
        xt = nc.dram_tensor("xt", [128 * k_chunks, MR], DT.float32r,
                            kind="ExternalInput")
    w = nc.dram_tensor("w", [128 * k_chunks, G], DT.float32r, kind="ExternalInput")
    bias = nc.dram_tensor("bias", [128, G], DT.float32, kind="ExternalInput")
    gi = nc.dram_tensor("gi", [MR, G], DT.float32, kind="ExternalOutput")
    n_m = MR // 128  # 64 m-chunks; m-chunk = 8 consecutive steps x 16 batch
    with TileContext(nc) as tc:
        with tc.tile_pool(name="wp", bufs=1) as wp, \
             tc.tile_pool(name="xp", bufs=3) as xp, \
             tc.tile_pool(name="op", bufs=2) as op, \
             tc.tile_pool(name="bp", bufs=1) as bp, \
             tc.tile_pool(name="ps", bufs=4, space="PSUM") as ps:
            wt = wp.tile([128, k_chunks, G], DT.float32r)
            nc.sync.dma_start(wt[:], w.rearrange("(k p) g -> p k g", p=128))
            bt = bp.tile([128, G], DT.float32)
            nc.sync.dma_start(bt[:], bias[:])
            for m in range(n_m):
                if from_ht:
                    x4 = xp.tile([128, k_chunks, 8, BQ], DT.float32r, tag="x")
                    c, r = divmod(m, n_m // 4)
                    nc.sync.dma_start(
                        x4[:], xts[c][8 * r:8 * r + 8].rearrange("t p k b -> p k t b"))
                    x = x4.rearrange("p k t b -> p k (t b)")
                else:
                    x = xp.tile([128, k_chunks, 128], DT.float32r, tag="x")
                    nc.sync.dma_start(x[:], xt.rearrange("(k p) m -> p k m", p=128)[
                        :, :, 128 * m:128 * (m + 1)])
                ot = op.tile([128, G], DT.float32, tag="o")
                for b in range(G // 512):
                    acc = ps.tile([128, 512], DT.float32, tag="acc")
                    for k in range(k_chunks):
                        nc.tensor.matmul(acc[:], x[:, k, :],
                                         wt[:, k, 512 * b:512 * (b + 1)],
                                         start=(k == 0), stop=(k == k_chunks - 1))
                    nc.vector.tensor_add(ot[:, 512 * b:512 * (b + 1)], acc[:],
                                         bt[:, 512 * b:512 * (b + 1)])
                nc.sync.dma_start(gi[128 * m:128 * (m + 1), :], ot[:])
    _split_multiwaits(nc)
    return nc


def build_scan(emit_ht, emit_h):
    """TCHUNK GRU steps for one cell on BQ batch rows.
    State: h [BQ, H] and its transpose ht [128, 8, BQ] (fp32r, PE stationary)."""
    T = TCHUNK
    nc = bass.Bass(trn_type="TRN2", num_devices=NCORES)
    whh = nc.dram_tensor("whh", [H, G], DT.float32r, kind="ExternalInput")
    gi = nc.dram_tensor("gi", [T, BQ, G], DT.float32, kind="ExternalInput")
    bhn = nc.dram_tensor("bhn", [BQ, H], DT.float32, kind="ExternalInput")
    h0 = nc.dram_tensor("h0", [BQ, H], DT.float32, kind="ExternalInput")
    ht0 = nc.dram_tensor("ht0", [128, 8, BQ], DT.float32r, kind="ExternalInput")
    ident = nc.dram_tensor("ident", [128, 128], DT.float32, kind="ExternalInput")
    hn = nc.dram_tensor("hn", [BQ, H], DT.float32, kind="ExternalOutput")
    htn = nc.dram_tensor("htn", [128, 8, BQ], DT.float32r, kind="ExternalOutput")
    if emit_ht:
        HT = nc.dram_tensor("HT", [T, 128, 8, BQ], DT.float32r,
                            kind="ExternalOutput")
    if emit_h:
        HO = nc.dram_tensor("HO", [T, BQ, H], DT.float32, kind="ExternalOutput")
    with TileContext(nc) as tc:
        with tc.tile_pool(name="wp", bufs=1) as wp, \
             tc.tile_pool(name="cp", bufs=1) as cp, \
             tc.tile_pool(name="gp", bufs=2) as gp, \
             tc.tile_pool(name="hp", bufs=2) as hp, \
             tc.tile_pool(name="tp", bufs=2) as tp, \
             tc.tile_pool(name="ep", bufs=1) as ep, \
             tc.tile_pool(name="ps", bufs=1, space="PSUM") as ps, \
             tc.tile_pool(name="pst", bufs=2, space="PSUM") as pst:
            wt = wp.tile([128, 8, G], DT.float32r)
            nc.sync.dma_start(wt[:], whh.rearrange("(k p) g -> p k g", p=128))
            bt = cp.tile([BQ, H], DT.float32, tag="bhn")
            nc.sync.dma_start(bt[:], bhn[:])
            idt = cp.tile([128, 128], DT.float32, tag="id")
            nc.sync.dma_start(idt[:], ident[:])
            h_prev = hp.tile([BQ, H], DT.float32, tag="h")
            nc.sync.dma_start(h_prev[:], h0[:])
            ht_prev = tp.tile([128, 8, BQ], DT.float32r, tag="ht")
            nc.sync.dma_start(ht_prev[:], ht0[:])
            for t in range(T):
                g = gp.tile([BQ, G], DT.float32, tag="gi")
                nc.sync.dma_start(g[:], gi[t])
                acc = ps.tile([BQ, G], DT.float32, tag="acc")
                # bank order: half-0's gate banks (r0,z0,n0) first so its
                # elementwise overlaps the remaining matmuls
                for b in (0, 2, 4, 1, 3, 5):
                    for k in range(8):
                        nc.tensor.matmul(acc[:, 512 * b:512 * (b + 1)],
                                         ht_prev[:, k, :],
                                         wt[:, k, 512 * b:512 * (b + 1)],
                                         start=(k == 0), stop=(k == 7))
                h_new = hp.tile([BQ, H], DT.float32, tag="h")
                ht_new = tp.tile([128, 8, BQ], DT.float32r, tag="ht")
                for j in (0, 1):
                    hs = slice(512 * j, 512 * (j + 1))        # hidden slice
                    rs = slice(512 * j, 512 * (j + 1))        # r cols
                    zs = slice(H + 512 * j, H + 512 * (j + 1))
                    ns = slice(2 * H + 512 * j, 2 * H + 512 * (j + 1))
                    pre = ep.tile([BQ, 1024], DT.float32, tag=f"pre{j}")
                    nc.vector.tensor_add(pre[:, 0:512], acc[:, rs], g[:, rs])
                    nc.vector.tensor_add(pre[:, 512:1024], acc[:, zs], g[:, zs])
                    rz = ep.tile([BQ, 1024], DT.float32, tag=f"rz{j}")
                    nc.scalar.activation(rz[:], pre[:], AF.Sigmoid)
                    hnb = ep.tile([BQ, 512], DT.float32, tag=f"hn{j}")
                    nc.vector.tensor_add(hnb[:], acc[:, ns], bt[:, hs])
                    rhn = ep.tile([BQ, 512], DT.float32, tag=f"rhn{j}")
                    nc.vector.tensor_mul(rhn[:], rz[:, 0:512], hnb[:])
                    npre = ep.tile([BQ, 512], DT.float32, tag=f"np{j}")
                    nc.vector.tensor_add(npre[:], g[:, ns], rhn[:])
                    nt = ep.tile([BQ, 512], DT.float32, tag=f"n{j}")
                    nc.scalar.activation(nt[:], npre[:], AF.Tanh)
                    d = ep.tile([BQ, 512], DT.float32, tag=f"d{j}")
                    nc.vector.tensor_sub(d[:], h_prev[:, hs], nt[:])
                    e = ep.tile([BQ, 512], DT.float32, tag=f"e{j}")
                    nc.vector.tensor_mul(e[:], rz[:, 512:1024], d[:])
                    nc.vector.tensor_add(h_new[:, hs], nt[:], e[:])
                    for c in range(4):
                        kk = 4 * j + c
                        tr = pst.tile([128, BQ], DT.float32, tag="tr")
                        nc.tensor.transpose(tr[:], h_new[:, 128 * kk:128 * (kk + 1)],
                                            idt[0:BQ, 0:BQ])
                        nc.vector.tensor_copy(ht_new[:, kk, :], tr[:])
                if emit_ht:
                    nc.sync.dma_start(HT[t], ht_new[:])
                if emit_h:
                    nc.sync.dma_start(HO[t], h_new[:])
                h_prev, ht_prev = h_new, ht_new
            nc.sync.dma_start(hn[:], h_prev[:])
            nc.sync.dma_start(htn[:], ht_prev[:])
    _split_multiwaits(nc)
    return nc


# ----------------------------------------------------------------- jax runner

_FNS = {}


def _make_runner(key, nc):
    """Sharded executor over 8 cores keeping arrays on device (modeled on
    bass2jax.run_bass_via_pjrt, minus the host round-trips)."""
    import jax
    import jax.numpy as jnp
    from jax.sharding import Mesh, PartitionSpec
    from jax.experimental.shard_map import shard_map
    from concourse import bass2jax as b2j

    b2j.install_neuronx_cc_hook()
    partition_name = (nc.partition_id_tensor.name
                      if nc.partition_id_tensor else None)
    in_names, out_names, out_avals = [], [], []
    for alloc in nc.m.functions[0].allocations:
        if not isinstance(alloc, mybir.MemoryLocationSet):
            continue
        name = alloc.memorylocations[0].name
        if alloc.kind == "ExternalInput":
            if name != partition_name:
                in_names.append(name)
        elif alloc.kind == "ExternalOutput":
            out_names.append(name)
            out_avals.append(jax.core.ShapedArray(
                tuple(alloc.tensor_shape), mybir.dt.np(alloc.dtype)))
    n_params = len(in_names)
    all_in = in_names + out_names + ([partition_name] if partition_name else [])

    def _body(*args):
        operands = list(args)
        if partition_name is not None:
            operands.append(b2j.partition_id_tensor())
        return tuple(b2j._bass_exec_p.bind(
            *operands, out_avals=tuple(out_avals), in_names=tuple(all_in),
            out_names=tuple(out_names), lowering_input_output_aliases=(),
            sim_require_finite=False, sim_require_nnan=False, nc=nc))

    devices = jax.devices()[:NCORES]
    mesh = Mesh(np.asarray(devices), ("core",))
    n_out = len(out_names)
    sharded = jax.jit(
        shard_map(_body, mesh=mesh,
                  in_specs=(PartitionSpec("core"),) * (n_params + n_out),
                  out_specs=(PartitionSpec("core"),) * n_out,
                  check_rep=False),
        donate_argnums=tuple(range(n_params, n_params + n_out)),
        keep_unused=True)

    def run(in_globals):
        """in_globals: dict name -> global array [8*d0, ...] (jax or np)."""
        import jax.numpy as jnp
        args = [in_globals[n] for n in in_names]
        zeros = [jnp.zeros((NCORES * a.shape[0], *a.shape[1:]), a.dtype)
                 for a in out_avals]
        outs = sharded(*args, *zeros)
        return dict(zip(out_names, outs))

    _FNS[key] = (run, in_names, out_names)
    return run


def _runner(key, builder):
    if key not in _FNS:
        _make_runner(key, builder())
    return _FNS[key][0]


# ----------------------------------------------------------------- host side


def _gather(per_core):
    """list of 8 per-core np arrays -> one global array (axis0 concat)."""
    return np.ascontiguousarray(np.concatenate(per_core, axis=0))


def kernel(x,
           w_ih0f, w_hh0f, b_ih0f, b_hh0f,
           w_ih0b, w_hh0b, b_ih0b, b_hh0b,
           w_ih1f, w_hh1f, b_ih1f, b_hh1f,
           w_ih1b, w_hh1b, b_ih1b, b_hh1b):
    import jax.numpy as jnp

    x = np.asarray(x, np.float32)
    f32 = lambda a: np.asarray(a, np.float32)
    # backward cell consumes x_t[:, ::-1]  ==  x_t @ flip(w_ih0b, axis=1)^T
    wih0 = {"f": f32(w_ih0f), "b": f32(w_ih0b)[:, ::-1]}
    whh0 = {"f": f32(w_hh0f), "b": f32(w_hh0b)}
    wih1 = {"f": f32(w_ih1f), "b": f32(w_ih1b)}
    whh1 = {"f": f32(w_hh1f), "b": f32(w_hh1b)}
    bias0 = {c: np.concatenate([
        (f32(bi)[:H] + f32(bh)[:H]),
        (f32(bi)[H:2 * H] + f32(bh)[H:2 * H]),
        f32(bi)[2 * H:]])
        for c, (bi, bh) in {"f": (b_ih0f, b_hh0f), "b": (b_ih0b, b_hh0b)}.items()}
    bias1 = {c: np.concatenate([
        (f32(bi)[:H] + f32(bh)[:H]),
        (f32(bi)[H:2 * H] + f32(bh)[H:2 * H]),
        f32(bi)[2 * H:]])
        for c, (bi, bh) in {"f": (b_ih1f, b_hh1f), "b": (b_ih1b, b_hh1b)}.items()}
    bhn0 = {c: f32(bh)[2 * H:] for c, bh in {"f": b_hh0f, "b": b_hh0b}.items()}
    bhn1 = {c: f32(bh)[2 * H:] for c, bh in {"f": b_hh1f, "b": b_hh1b}.items()}

    cores = [("f", q) for q in range(4)] + [("b", q) for q in range(4)]

    # --- phase A inputs: XT [IN, S*BQ] per core, W [IN, G], bias rep [128, G]
    xt_pc, wA_pc, biasA_pc = [], [], []
    for c, q in cores:
        xq = x[BQ * q:BQ * (q + 1)]                     # [BQ, S, IN]
        xt_pc.append(np.ascontiguousarray(
            xq.transpose(2, 1, 0).reshape(IN, S * BQ)))  # [(IN), (t b)]
        wA_pc.append(np.ascontiguousarray(wih0[c].T))    # [IN, G]
        biasA_pc.append(np.broadcast_to(bias0[c], (128, G)).copy())
    runA = _runner("gemmA", lambda: build_gemm(IN // 128, False))
    outA = runA({"xt": _gather(xt_pc), "w": _gather(wA_pc),
                 "bias": _gather(biasA_pc)})
    gi0 = outA["gi"]                                     # [8*8192, G] on device

    # --- scan 0
    runS0 = _runner("scan_ht", lambda: build_scan(True, False))
    whh0_pc = _gather([np.ascontiguousarray(whh0[c].T) for c, q in cores])
    bhn0_pc = _gather([np.broadcast_to(bhn0[c], (BQ, H)).copy()
                       for c, q in cores])
    ident = _gather([np.eye(128, dtype=np.float32)] * NCORES)
    h = jnp.zeros((NCORES * BQ, H), np.float32)
    ht = jnp.zeros((NCORES * 128, 8, BQ), np.float32)
    gi0_v = gi0.reshape(NCORES, S, BQ, G)
    hts = []
    for ch in range(S // TCHUNK):
        gi_sl = gi0_v[:, TCHUNK * ch:TCHUNK * (ch + 1)].reshape(
            NCORES * TCHUNK, BQ, G)
        o = runS0({"whh": whh0_pc, "gi": gi_sl, "bhn": bhn0_pc, "h0": h,
                   "ht0": ht, "ident": ident})
        h, ht = o["hn"], o["htn"]
        hts.append(o["HT"])

    # --- phase C: gi1 from scan-0 states
    runC = _runner("gemmC", lambda: build_gemm(H // 128, True))
    inC = {f"xt{i}": hts[i] for i in range(len(hts))}
    inC["w"] = _gather([np.ascontiguousarray(wih1[c].T) for c, q in cores])
    inC["bias"] = _gather([np.broadcast_to(bias1[c], (128, G)).copy()
                           for c, q in cores])
    gi1 = runC(inC)["gi"]

    # --- scan 1
    runS1 = _runner("scan_h", lambda: build_scan(False, True))
    whh1_pc = _gather([np.ascontiguousarray(whh1[c].T) for c, q in cores])
    bhn1_pc = _gather([np.broadcast_to(bhn1[c], (BQ, H)).copy()
                       for c, q in cores])
    h = jnp.zeros((NCORES * BQ, H), np.float32)
    ht = jnp.zeros((NCORES * 128, 8, BQ), np.float32)
    gi1_v = gi1.reshape(NCORES, S, BQ, G)
    hos = []
    for ch in range(S // TCHUNK):
        gi_sl = gi1_v[:, TCHUNK * ch:TCHUNK * (ch + 1)].reshape(
            NCORES * TCHUNK, BQ, G)
        o = runS1({"whh": whh1_pc, "gi": gi_sl, "bhn": bhn1_pc, "h0": h,
                   "ht0": ht, "ident": ident})
        h, ht = o["hn"], o["htn"]
        hos.append(o["HO"])

    # --- assemble [B, S, 2H]
    h1 = np.stack([np.asarray(o).reshape(NCORES, TCHUNK, BQ, H)
                   for o in hos], axis=1)                # [core, chunk, t, b, H]
    h1 = h1.transpose(0, 3, 1, 2, 4).reshape(NCORES, BQ, S, H)
    out = np.empty((B, S, 2 * H), np.float32)
    for i, (c, q) in enumerate(cores):
        col = slice(0, H) if c == "f" else slice(H, 2 * H)
        out[BQ * q:BQ * (q + 1), :, col] = h1[i]
    return out, np.ascontiguousarray(out[:, -1, :])


# revision 10
# speedup vs baseline: 1.8683x; 1.8683x over previous
"""Bidirectional 2-layer GRU encoder on 8 Trainium2 NeuronCores.

B=64, S=512, IN=512, H=1024. The reference's "backward" direction is a
feature-axis flip of x_t (not time reversal), so all 4 GRU cells scan forward
in time. Sharding: 8 cores = 2 cells (f/b chain) x 4 batch quarters (16 rows
each); each core runs its chain completely locally (no cross-core comm):

  phase A  : gi0 = Xq @ w_ih0^T + bias           (big GEMM, per core)
  scan 0   : layer-0 GRU scan, 512 steps          (emits transposed h states)
  phase C  : gi1 = H0q @ w_ih1^T + bias           (big GEMM from scan-0 states)
  scan 1   : layer-1 GRU scan, 512 steps          (emits h1 sequence)

Recurrent matmuls run with the batch-transposed state as the PE stationary
operand and w_hh^T streaming (fp32r: 1 cycle/row at N=512). Biases are
pre-folded into gi (b_ih for all gates + b_hh for r,z); b_hh_n is added on
the h-side inside the scan, matching PyTorch GRU gate math exactly.
"""

import os
import sys
import numpy as np

import concourse.bass as bass
import concourse.mybir as mybir
from concourse.tile import TileContext
from concourse.vector_clock import ScopedClock

B, S, IN, H = 64, 512, 512, 1024
G = 3 * H            # 3072 gate columns, order [r | z | n]
NCORES = 8
BQ = B // 4          # 16 batch rows per core
TCHUNK = 512         # scan steps per launch
AF = mybir.ActivationFunctionType
DT = mybir.dt

# ----------------------------------------------------------------- walrus fixes


def _patched_drain_and_barrier(self, tick_clock, wait_clock):
    nc = self.nc
    probe = nc.sync.nop(nofuse=True)
    wait_clock.add_sem_waits(probe.ins, ScopedClock({None: tick_clock.global_clock}))
    si = probe.ins.sync_info
    waits = list(si.on_wait) if si is not None else []
    probe.ins.sync_info = mybir.SyncInfo(on_wait=waits[:1], on_update=[])
    for w in waits[1:]:
        n2 = nc.sync.nop(nofuse=True)
        n2.ins.sync_info = mybir.SyncInfo(on_wait=[w], on_update=[])
    nc.sync.drain()
    nc.all_engine_barrier()
    popped = nc._tile_sem_poison_stack.pop()
    assert popped is self._sem_poison
    nc.clear_and_free_semaphores(list(self.sems.allocated().values()))
    nc.all_engine_barrier()


TileContext._drain_and_barrier = _patched_drain_and_barrier


def _split_multiwaits(nc):
    """This container's walrus accepts at most one sync-wait per instruction;
    hoist extras onto same-engine NOPs (sequencers are strict FIFO)."""
    for f in nc.m.functions:
        for bb in f.blocks:
            insts = bb.instructions
            i = 0
            while i < len(insts):
                inst = insts[i]
                si = inst.sync_info
                if si is not None and len(si.on_wait) > 1:
                    waits = list(si.on_wait)
                    for j, w in enumerate(waits[:-1]):
                        nop = mybir.InstNoOp(
                            name=f"{inst.name}.wsplit{j}", engine=inst.engine,
                            sync_info=mybir.SyncInfo(on_wait=[w], on_update=[]),
                            bass_nofuse=True)
                        insts.insert(i, nop)
                        i += 1
                    inst.sync_info = mybir.SyncInfo(
                        on_wait=[waits[-1]], on_update=list(si.on_update))
                i += 1


# ----------------------------------------------------------------- kernel builders


def build_gemm(k_chunks, from_ht):
    """gi[MR, G] = XT^T @ W + bias.  XT is the pre-transposed left operand
    [128*k_chunks, MR] (from_ht=False, one DRAM tensor) or four scan-chunk
    state dumps [TCHUNK, 128, 8, 16] (from_ht=True). W is [128*k_chunks, G].
    bias is replicated [128, G]. Row m of gi is (t, b) = (m // 16, m % 16)."""
    MR = S * BQ  # 8192
    nc = bass.Bass(trn_type="TRN2", num_devices=NCORES)
    if from_ht:
        xts = [nc.dram_tensor(f"xt{i}", [TCHUNK, 128, 8, BQ], DT.float32r,
                              kind="ExternalInput") for i in range(S // TCHUNK)]
    else:
        xt = nc.dram_tensor("xt", [128 * k_chunks, MR], DT.float32r,
                            kind="ExternalInput")
    w = nc.dram_tensor("w", [128 * k_chunks, G], DT.float32r, kind="ExternalInput")
    bias = nc.dram_tensor("bias", [128, G], DT.float32, kind="ExternalInput")
    gi = nc.dram_tensor("gi", [MR, G], DT.float32, kind="ExternalOutput")
    n_m = MR // 128  # 64 m-chunks; m-chunk = 8 consecutive steps x 16 batch
    with TileContext(nc) as tc:
        with tc.tile_pool(name="wp", bufs=1) as wp, \
             tc.tile_pool(name="xp", bufs=3) as xp, \
             tc.tile_pool(name="op", bufs=2) as op, \
             tc.tile_pool(name="bp", bufs=1) as bp, \
             tc.tile_pool(name="ps", bufs=4, space="PSUM") as ps:
            wt = wp.tile([128, k_chunks, G], DT.float32r)
            nc.sync.dma_start(wt[:], w.rearrange("(k p) g -> p k g", p=128))
            bt = bp.tile([128, G], DT.float32)
            nc.sync.dma_start(bt[:], bias[:])
            for m in range(n_m):
                if from_ht:
                    x4 = xp.tile([128, k_chunks, 8, BQ], DT.float32r, tag="x")
                    c, r = divmod(m, n_m // len(xts))
                    nc.sync.dma_start(
                        x4[:], xts[c][8 * r:8 * r + 8].rearrange("t p k b -> p k t b"))
                    x = x4.rearrange("p k t b -> p k (t b)")
                else:
                    x = xp.tile([128, k_chunks, 128], DT.float32r, tag="x")
                    nc.sync.dma_start(x[:], xt.rearrange("(k p) m -> p k m", p=128)[
                        :, :, 128 * m:128 * (m + 1)])
                ot = op.tile([128, G], DT.float32, tag="o")
                for b in range(G // 512):
                    acc = ps.tile([128, 512], DT.float32, tag="acc")
                    for k in range(k_chunks):
                        nc.tensor.matmul(acc[:], x[:, k, :],
                                         wt[:, k, 512 * b:512 * (b + 1)],
                                         start=(k == 0), stop=(k == k_chunks - 1))
                    nc.vector.tensor_add(ot[:, 512 * b:512 * (b + 1)], acc[:],
                                         bt[:, 512 * b:512 * (b + 1)])
                nc.sync.dma_start(gi[128 * m:128 * (m + 1), :], ot[:])
    _split_multiwaits(nc)
    return nc


def build_scan(emit_ht, emit_h):
    """TCHUNK GRU steps for one cell on BQ batch rows.
    State: h [BQ, H] and its transpose ht [128, 8, BQ] (fp32r, PE stationary)."""
    T = TCHUNK
    nc = bass.Bass(trn_type="TRN2", num_devices=NCORES)
    whh = nc.dram_tensor("whh", [H, G], DT.float32r, kind="ExternalInput")
    gi = nc.dram_tensor("gi", [T, BQ, G], DT.float32, kind="ExternalInput")
    bhn = nc.dram_tensor("bhn", [BQ, H], DT.float32, kind="ExternalInput")
    h0 = nc.dram_tensor("h0", [BQ, H], DT.float32, kind="ExternalInput")
    ht0 = nc.dram_tensor("ht0", [128, 8, BQ], DT.float32r, kind="ExternalInput")
    ident = nc.dram_tensor("ident", [128, 128], DT.float32, kind="ExternalInput")
    hn = nc.dram_tensor("hn", [BQ, H], DT.float32, kind="ExternalOutput")
    htn = nc.dram_tensor("htn", [128, 8, BQ], DT.float32r, kind="ExternalOutput")
    if emit_ht:
        HT = nc.dram_tensor("HT", [T, 128, 8, BQ], DT.float32r,
                            kind="ExternalOutput")
    if emit_h:
        HO = nc.dram_tensor("HO", [T, BQ, H], DT.float32, kind="ExternalOutput")
    with TileContext(nc) as tc:
        with tc.tile_pool(name="wp", bufs=1) as wp, \
             tc.tile_pool(name="cp", bufs=1) as cp, \
             tc.tile_pool(name="gp", bufs=2) as gp, \
             tc.tile_pool(name="hp", bufs=2) as hp, \
             tc.tile_pool(name="tp", bufs=2) as tp, \
             tc.tile_pool(name="ep", bufs=1) as ep, \
             tc.tile_pool(name="ps", bufs=1, space="PSUM") as ps, \
             tc.tile_pool(name="pst", bufs=2, space="PSUM") as pst:
            wt = wp.tile([128, 8, G], DT.float32r)
            nc.sync.dma_start(wt[:], whh.rearrange("(k p) g -> p k g", p=128))
            bt = cp.tile([BQ, H], DT.float32, tag="bhn")
            nc.sync.dma_start(bt[:], bhn[:])
            idt = cp.tile([128, 128], DT.float32, tag="id")
            nc.sync.dma_start(idt[:], ident[:])
            h_prev = hp.tile([BQ, H], DT.float32, tag="h")
            nc.sync.dma_start(h_prev[:], h0[:])
            ht_prev = tp.tile([128, 8, BQ], DT.float32r, tag="ht")
            nc.sync.dma_start(ht_prev[:], ht0[:])
            for t in range(T):
                g = gp.tile([BQ, G], DT.float32, tag="gi")
                nc.sync.dma_start(g[:], gi[t])
                acc = ps.tile([BQ, G], DT.float32, tag="acc")
                # bank order: half-0's gate banks (r0,z0,n0) first so its
                # elementwise overlaps the remaining matmuls
                for b in (0, 2, 4, 1, 3, 5):
                    for k in range(8):
                        nc.tensor.matmul(acc[:, 512 * b:512 * (b + 1)],
                                         ht_prev[:, k, :],
                                         wt[:, k, 512 * b:512 * (b + 1)],
                                         start=(k == 0), stop=(k == 7))
                h_new = hp.tile([BQ, H], DT.float32, tag="h")
                ht_new = tp.tile([128, 8, BQ], DT.float32r, tag="ht")
                for j in (0, 1):
                    hs = slice(512 * j, 512 * (j + 1))        # hidden slice
                    rs = slice(512 * j, 512 * (j + 1))        # r cols
                    zs = slice(H + 512 * j, H + 512 * (j + 1))
                    ns = slice(2 * H + 512 * j, 2 * H + 512 * (j + 1))
                    pre = ep.tile([BQ, 1024], DT.float32, tag=f"pre{j}")
                    nc.vector.tensor_add(pre[:, 0:512], acc[:, rs], g[:, rs])
                    nc.vector.tensor_add(pre[:, 512:1024], acc[:, zs], g[:, zs])
                    rz = ep.tile([BQ, 1024], DT.float32, tag=f"rz{j}")
                    nc.scalar.activation(rz[:], pre[:], AF.Sigmoid)
                    hnb = ep.tile([BQ, 512], DT.float32, tag=f"hn{j}")
                    nc.vector.tensor_add(hnb[:], acc[:, ns], bt[:, hs])
                    rhn = ep.tile([BQ, 512], DT.float32, tag=f"rhn{j}")
                    nc.vector.tensor_mul(rhn[:], rz[:, 0:512], hnb[:])
                    npre = ep.tile([BQ, 512], DT.float32, tag=f"np{j}")
                    nc.vector.tensor_add(npre[:], g[:, ns], rhn[:])
                    nt = ep.tile([BQ, 512], DT.float32, tag=f"n{j}")
                    nc.scalar.activation(nt[:], npre[:], AF.Tanh)
                    d = ep.tile([BQ, 512], DT.float32, tag=f"d{j}")
                    nc.vector.tensor_sub(d[:], h_prev[:, hs], nt[:])
                    e = ep.tile([BQ, 512], DT.float32, tag=f"e{j}")
                    nc.vector.tensor_mul(e[:], rz[:, 512:1024], d[:])
                    nc.vector.tensor_add(h_new[:, hs], nt[:], e[:])
                    for c in range(4):
                        kk = 4 * j + c
                        tr = pst.tile([128, BQ], DT.float32, tag="tr")
                        nc.tensor.transpose(tr[:], h_new[:, 128 * kk:128 * (kk + 1)],
                                            idt[0:BQ, 0:BQ])
                        nc.vector.tensor_copy(ht_new[:, kk, :], tr[:])
                if emit_ht:
                    nc.sync.dma_start(HT[t], ht_new[:])
                if emit_h:
                    nc.sync.dma_start(HO[t], h_new[:])
                h_prev, ht_prev = h_new, ht_new
            nc.sync.dma_start(hn[:], h_prev[:])
            nc.sync.dma_start(htn[:], ht_prev[:])
    _split_multiwaits(nc)
    return nc


# ----------------------------------------------------------------- jax runner

_FNS = {}
_MESH = [None]


def _fetch(garr):
    """Fetch a sharded global array pulling the 8 per-device shards in
    parallel (the tunnel serializes single-stream fetches)."""
    from concurrent.futures import ThreadPoolExecutor
    shards = list(garr.addressable_shards)
    shards.sort(key=lambda sh: (sh.index[0].start or 0) if sh.index else 0)
    with ThreadPoolExecutor(len(shards)) as ex:
        parts = list(ex.map(lambda sh: np.asarray(sh.data), shards))
    return np.concatenate(parts, axis=0)


def _to_dev(arr):
    """Place a global [8*d0, ...] array on the 8-core mesh once."""
    import jax
    from jax.sharding import NamedSharding, PartitionSpec
    return jax.device_put(arr, NamedSharding(_MESH[0], PartitionSpec("core")))


def _make_runner(key, nc):
    """Sharded executor over 8 cores keeping arrays on device (modeled on
    bass2jax.run_bass_via_pjrt, minus the host round-trips)."""
    import jax
    import jax.numpy as jnp
    from jax.sharding import Mesh, PartitionSpec
    from jax.experimental.shard_map import shard_map
    from concourse import bass2jax as b2j

    b2j.install_neuronx_cc_hook()
    partition_name = (nc.partition_id_tensor.name
                      if nc.partition_id_tensor else None)
    in_names, out_names, out_avals = [], [], []
    for alloc in nc.m.functions[0].allocations:
        if not isinstance(alloc, mybir.MemoryLocationSet):
            continue
        name = alloc.memorylocations[0].name
        if alloc.kind == "ExternalInput":
            if name != partition_name:
                in_names.append(name)
        elif alloc.kind == "ExternalOutput":
            out_names.append(name)
            out_avals.append(jax.core.ShapedArray(
                tuple(alloc.tensor_shape), mybir.dt.np(alloc.dtype)))
    n_params = len(in_names)
    all_in = in_names + out_names + ([partition_name] if partition_name else [])

    def _body(*args):
        operands = list(args)
        if partition_name is not None:
            operands.append(b2j.partition_id_tensor())
        return tuple(b2j._bass_exec_p.bind(
            *operands, out_avals=tuple(out_avals), in_names=tuple(all_in),
            out_names=tuple(out_names), lowering_input_output_aliases=(),
            sim_require_finite=False, sim_require_nnan=False, nc=nc))

    devices = jax.devices()[:NCORES]
    mesh = Mesh(np.asarray(devices), ("core",))
    _MESH[0] = mesh
    n_out = len(out_names)
    sharded = jax.jit(
        shard_map(_body, mesh=mesh,
                  in_specs=(PartitionSpec("core"),) * (n_params + n_out),
                  out_specs=(PartitionSpec("core"),) * n_out,
                  check_rep=False),
        donate_argnums=tuple(range(n_params, n_params + n_out)),
        keep_unused=True)

    def run(in_globals):
        """in_globals: dict name -> global array [8*d0, ...] (jax or np)."""
        import jax.numpy as jnp
        args = [in_globals[n] for n in in_names]
        zeros = [jnp.zeros((NCORES * a.shape[0], *a.shape[1:]), a.dtype)
                 for a in out_avals]
        outs = sharded(*args, *zeros)
        return dict(zip(out_names, outs))

    _FNS[key] = (run, in_names, out_names)
    return run


def _runner(key, builder):
    if key not in _FNS:
        _make_runner(key, builder())
    return _FNS[key][0]


# ----------------------------------------------------------------- host side


def _gather(per_core):
    """list of 8 per-core np arrays -> one global array (axis0 concat)."""
    return np.ascontiguousarray(np.concatenate(per_core, axis=0))


VERBOSE = os.environ.get("GRU_VERBOSE", "0") == "1"


def _tick(label, t0=[None]):
    import time
    now = time.time()
    if VERBOSE and t0[0] is not None:
        print(f"  [kernel] {label}: {now - t0[0]:.2f}s", flush=True)
    t0[0] = now


def kernel(x,
           w_ih0f, w_hh0f, b_ih0f, b_hh0f,
           w_ih0b, w_hh0b, b_ih0b, b_hh0b,
           w_ih1f, w_hh1f, b_ih1f, b_hh1f,
           w_ih1b, w_hh1b, b_ih1b, b_hh1b):
    import jax.numpy as jnp

    x = np.asarray(x, np.float32)
    f32 = lambda a: np.asarray(a, np.float32)
    # backward cell consumes x_t[:, ::-1]  ==  x_t @ flip(w_ih0b, axis=1)^T
    wih0 = {"f": f32(w_ih0f), "b": f32(w_ih0b)[:, ::-1]}
    whh0 = {"f": f32(w_hh0f), "b": f32(w_hh0b)}
    wih1 = {"f": f32(w_ih1f), "b": f32(w_ih1b)}
    whh1 = {"f": f32(w_hh1f), "b": f32(w_hh1b)}
    bias0 = {c: np.concatenate([
        (f32(bi)[:H] + f32(bh)[:H]),
        (f32(bi)[H:2 * H] + f32(bh)[H:2 * H]),
        f32(bi)[2 * H:]])
        for c, (bi, bh) in {"f": (b_ih0f, b_hh0f), "b": (b_ih0b, b_hh0b)}.items()}
    bias1 = {c: np.concatenate([
        (f32(bi)[:H] + f32(bh)[:H]),
        (f32(bi)[H:2 * H] + f32(bh)[H:2 * H]),
        f32(bi)[2 * H:]])
        for c, (bi, bh) in {"f": (b_ih1f, b_hh1f), "b": (b_ih1b, b_hh1b)}.items()}
    bhn0 = {c: f32(bh)[2 * H:] for c, bh in {"f": b_hh0f, "b": b_hh0b}.items()}
    bhn1 = {c: f32(bh)[2 * H:] for c, bh in {"f": b_hh1f, "b": b_hh1b}.items()}

    cores = [("f", q) for q in range(4)] + [("b", q) for q in range(4)]

    # --- phase A inputs: XT [IN, S*BQ] per core, W [IN, G], bias rep [128, G]
    xt_pc, wA_pc, biasA_pc = [], [], []
    for c, q in cores:
        xq = x[BQ * q:BQ * (q + 1)]                     # [BQ, S, IN]
        xt_pc.append(np.ascontiguousarray(
            xq.transpose(2, 1, 0).reshape(IN, S * BQ)))  # [(IN), (t b)]
        wA_pc.append(np.ascontiguousarray(wih0[c].T))    # [IN, G]
        biasA_pc.append(np.broadcast_to(bias0[c], (128, G)).copy())
    _tick("prep")
    runA = _runner("gemmA", lambda: build_gemm(IN // 128, False))
    outA = runA({"xt": _gather(xt_pc), "w": _gather(wA_pc),
                 "bias": _gather(biasA_pc)})
    _tick("gemmA")
    gi0 = outA["gi"]                                     # [8*8192, G] on device

    # --- scan 0
    runS0 = _runner("scan_ht", lambda: build_scan(True, False))
    whh0_pc = _to_dev(_gather([np.ascontiguousarray(whh0[c].T) for c, q in cores]))
    bhn0_pc = _to_dev(_gather([np.broadcast_to(bhn0[c], (BQ, H)).copy()
                               for c, q in cores]))
    ident = _to_dev(_gather([np.eye(128, dtype=np.float32)] * NCORES))
    h = jnp.zeros((NCORES * BQ, H), np.float32)
    ht = jnp.zeros((NCORES * 128, 8, BQ), np.float32)
    gi0_v = gi0.reshape(NCORES, S, BQ, G)
    hts = []
    for ch in range(S // TCHUNK):
        gi_sl = gi0_v[:, TCHUNK * ch:TCHUNK * (ch + 1)].reshape(
            NCORES * TCHUNK, BQ, G)
        o = runS0({"whh": whh0_pc, "gi": gi_sl, "bhn": bhn0_pc, "h0": h,
                   "ht0": ht, "ident": ident})
        h, ht = o["hn"], o["htn"]
        hts.append(o["HT"])

    _tick("scan0")
    # --- phase C: gi1 from scan-0 states
    runC = _runner("gemmC", lambda: build_gemm(H // 128, True))
    inC = {f"xt{i}": hts[i] for i in range(len(hts))}
    inC["w"] = _gather([np.ascontiguousarray(wih1[c].T) for c, q in cores])
    inC["bias"] = _gather([np.broadcast_to(bias1[c], (128, G)).copy()
                           for c, q in cores])
    gi1 = runC(inC)["gi"]

    _tick("gemmC")
    # --- scan 1
    runS1 = _runner("scan_h", lambda: build_scan(False, True))
    whh1_pc = _to_dev(_gather([np.ascontiguousarray(whh1[c].T) for c, q in cores]))
    bhn1_pc = _to_dev(_gather([np.broadcast_to(bhn1[c], (BQ, H)).copy()
                               for c, q in cores]))
    h = jnp.zeros((NCORES * BQ, H), np.float32)
    ht = jnp.zeros((NCORES * 128, 8, BQ), np.float32)
    gi1_v = gi1.reshape(NCORES, S, BQ, G)
    hos = []
    for ch in range(S // TCHUNK):
        gi_sl = gi1_v[:, TCHUNK * ch:TCHUNK * (ch + 1)].reshape(
            NCORES * TCHUNK, BQ, G)
        o = runS1({"whh": whh1_pc, "gi": gi_sl, "bhn": bhn1_pc, "h0": h,
                   "ht0": ht, "ident": ident})
        h, ht = o["hn"], o["htn"]
        hos.append(o["HO"])

    _tick("scan1")
    # --- assemble [B, S, 2H]
    import jax.numpy as _jnp
    h1_dev = [_jnp.transpose(o.reshape(NCORES, TCHUNK, BQ, H), (0, 2, 1, 3))
              for o in hos]                              # [core, b, t, H] chunks
    h1 = np.concatenate([np.asarray(o) for o in h1_dev], axis=2) \
        if len(h1_dev) > 1 else np.asarray(h1_dev[0])    # [core, b, S, H]
    out = np.empty((B, S, 2 * H), np.float32)
    for i, (c, q) in enumerate(cores):
        col = slice(0, H) if c == "f" else slice(H, 2 * H)
        out[BQ * q:BQ * (q + 1), :, col] = h1[i]
    _tick("assemble")
    return out, np.ascontiguousarray(out[:, -1, :])


# revision 11
# speedup vs baseline: 1545.6130x; 827.2948x over previous
"""Bidirectional 2-layer GRU encoder on 8 Trainium2 NeuronCores.

B=64, S=512, IN=512, H=1024. The reference's "backward" direction is a
feature-axis flip of x_t (not time reversal), so all 4 GRU cells scan forward
in time. Sharding: 8 cores = 2 cells (f/b chain) x 4 batch quarters (16 rows
each); each core runs its chain completely locally (no cross-core comm):

  phase A  : gi0 = Xq @ w_ih0^T + bias           (big GEMM, per core)
  scan 0   : layer-0 GRU scan, 512 steps          (emits transposed h states)
  phase C  : gi1 = H0q @ w_ih1^T + bias           (big GEMM from scan-0 states)
  scan 1   : layer-1 GRU scan, 512 steps          (emits h1 sequence)

Recurrent matmuls run with the batch-transposed state as the PE stationary
operand and w_hh^T streaming (fp32r: 1 cycle/row at N=512). Biases are
pre-folded into gi (b_ih for all gates + b_hh for r,z); b_hh_n is added on
the h-side inside the scan, matching PyTorch GRU gate math exactly.
"""

import os
import sys
import numpy as np

import concourse.bass as bass
import concourse.mybir as mybir
from concourse.tile import TileContext
from concourse.vector_clock import ScopedClock

B, S, IN, H = 64, 512, 512, 1024
G = 3 * H            # 3072 gate columns, order [r | z | n]
NCORES = 8
BQ = B // 4          # 16 batch rows per core
TCHUNK = 512         # scan steps per launch
AF = mybir.ActivationFunctionType
DT = mybir.dt

# ----------------------------------------------------------------- walrus fixes


def _patched_drain_and_barrier(self, tick_clock, wait_clock):
    nc = self.nc
    probe = nc.sync.nop(nofuse=True)
    wait_clock.add_sem_waits(probe.ins, ScopedClock({None: tick_clock.global_clock}))
    si = probe.ins.sync_info
    waits = list(si.on_wait) if si is not None else []
    probe.ins.sync_info = mybir.SyncInfo(on_wait=waits[:1], on_update=[])
    for w in waits[1:]:
        n2 = nc.sync.nop(nofuse=True)
        n2.ins.sync_info = mybir.SyncInfo(on_wait=[w], on_update=[])
    nc.sync.drain()
    nc.all_engine_barrier()
    popped = nc._tile_sem_poison_stack.pop()
    assert popped is self._sem_poison
    nc.clear_and_free_semaphores(list(self.sems.allocated().values()))
    nc.all_engine_barrier()


TileContext._drain_and_barrier = _patched_drain_and_barrier


def _split_multiwaits(nc):
    """This container's walrus accepts at most one sync-wait per instruction;
    hoist extras onto same-engine NOPs (sequencers are strict FIFO)."""
    for f in nc.m.functions:
        for bb in f.blocks:
            insts = bb.instructions
            i = 0
            while i < len(insts):
                inst = insts[i]
                si = inst.sync_info
                if si is not None and len(si.on_wait) > 1:
                    waits = list(si.on_wait)
                    for j, w in enumerate(waits[:-1]):
                        nop = mybir.InstNoOp(
                            name=f"{inst.name}.wsplit{j}", engine=inst.engine,
                            sync_info=mybir.SyncInfo(on_wait=[w], on_update=[]),
                            bass_nofuse=True)
                        insts.insert(i, nop)
                        i += 1
                    inst.sync_info = mybir.SyncInfo(
                        on_wait=[waits[-1]], on_update=list(si.on_update))
                i += 1


# ----------------------------------------------------------------- kernel builders


def build_gemm(k_chunks, from_ht):
    """gi[MR, G] = XT^T @ W + bias.  XT is the pre-transposed left operand
    [128*k_chunks, MR] (from_ht=False, one DRAM tensor) or four scan-chunk
    state dumps [TCHUNK, 128, 8, 16] (from_ht=True). W is [128*k_chunks, G].
    bias is replicated [128, G]. Row m of gi is (t, b) = (m // 16, m % 16)."""
    MR = S * BQ  # 8192
    nc = bass.Bass(trn_type="TRN2", num_devices=NCORES)
    if from_ht:
        xts = [nc.dram_tensor(f"xt{i}", [TCHUNK, 128, 8, BQ], DT.float32r,
                              kind="ExternalInput") for i in range(S // TCHUNK)]
    else:
        xt = nc.dram_tensor("xt", [128 * k_chunks, MR], DT.float32r,
                            kind="ExternalInput")
    w = nc.dram_tensor("w", [128 * k_chunks, G], DT.float32r, kind="ExternalInput")
    bias = nc.dram_tensor("bias", [128, G], DT.float32, kind="ExternalInput")
    gi = nc.dram_tensor("gi", [MR, G], DT.float32, kind="ExternalOutput")
    n_m = MR // 128  # 64 m-chunks; m-chunk = 8 consecutive steps x 16 batch
    with TileContext(nc) as tc:
        with tc.tile_pool(name="wp", bufs=1) as wp, \
             tc.tile_pool(name="xp", bufs=3) as xp, \
             tc.tile_pool(name="op", bufs=2) as op, \
             tc.tile_pool(name="bp", bufs=1) as bp, \
             tc.tile_pool(name="ps", bufs=4, space="PSUM") as ps:
            wt = wp.tile([128, k_chunks, G], DT.float32r)
            nc.sync.dma_start(wt[:], w.rearrange("(k p) g -> p k g", p=128))
            bt = bp.tile([128, G], DT.float32)
            nc.sync.dma_start(bt[:], bias[:])
            for m in range(n_m):
                if from_ht:
                    x4 = xp.tile([128, k_chunks, 8, BQ], DT.float32r, tag="x")
                    c, r = divmod(m, n_m // len(xts))
                    nc.sync.dma_start(
                        x4[:], xts[c][8 * r:8 * r + 8].rearrange("t p k b -> p k t b"))
                    x = x4.rearrange("p k t b -> p k (t b)")
                else:
                    x = xp.tile([128, k_chunks, 128], DT.float32r, tag="x")
                    nc.sync.dma_start(x[:], xt.rearrange("(k p) m -> p k m", p=128)[
                        :, :, 128 * m:128 * (m + 1)])
                ot = op.tile([128, G], DT.float32, tag="o")
                for b in range(G // 512):
                    acc = ps.tile([128, 512], DT.float32, tag="acc")
                    for k in range(k_chunks):
                        nc.tensor.matmul(acc[:], x[:, k, :],
                                         wt[:, k, 512 * b:512 * (b + 1)],
                                         start=(k == 0), stop=(k == k_chunks - 1))
                    nc.vector.tensor_add(ot[:, 512 * b:512 * (b + 1)], acc[:],
                                         bt[:, 512 * b:512 * (b + 1)])
                nc.sync.dma_start(gi[128 * m:128 * (m + 1), :], ot[:])
    _split_multiwaits(nc)
    return nc


def build_scan(emit_ht, emit_h):
    """TCHUNK GRU steps for one cell on BQ batch rows.
    State: h [BQ, H] and its transpose ht [128, 8, BQ] (fp32r, PE stationary)."""
    T = TCHUNK
    nc = bass.Bass(trn_type="TRN2", num_devices=NCORES)
    whh = nc.dram_tensor("whh", [H, G], DT.float32r, kind="ExternalInput")
    gi = nc.dram_tensor("gi", [T, BQ, G], DT.float32, kind="ExternalInput")
    bhn = nc.dram_tensor("bhn", [BQ, H], DT.float32, kind="ExternalInput")
    h0 = nc.dram_tensor("h0", [BQ, H], DT.float32, kind="ExternalInput")
    ht0 = nc.dram_tensor("ht0", [128, 8, BQ], DT.float32r, kind="ExternalInput")
    ident = nc.dram_tensor("ident", [128, 128], DT.float32, kind="ExternalInput")
    hn = nc.dram_tensor("hn", [BQ, H], DT.float32, kind="ExternalOutput")
    htn = nc.dram_tensor("htn", [128, 8, BQ], DT.float32r, kind="ExternalOutput")
    if emit_ht:
        HT = nc.dram_tensor("HT", [T, 128, 8, BQ], DT.float32r,
                            kind="ExternalOutput")
    if emit_h:
        HO = nc.dram_tensor("HO", [T, BQ, H], DT.float32, kind="ExternalOutput")
    with TileContext(nc) as tc:
        with tc.tile_pool(name="wp", bufs=1) as wp, \
             tc.tile_pool(name="cp", bufs=1) as cp, \
             tc.tile_pool(name="gp", bufs=2) as gp, \
             tc.tile_pool(name="hp", bufs=2) as hp, \
             tc.tile_pool(name="tp", bufs=2) as tp, \
             tc.tile_pool(name="ep", bufs=1) as ep, \
             tc.tile_pool(name="ps", bufs=1, space="PSUM") as ps, \
             tc.tile_pool(name="pst", bufs=2, space="PSUM") as pst:
            wt = wp.tile([128, 8, G], DT.float32r)
            nc.sync.dma_start(wt[:], whh.rearrange("(k p) g -> p k g", p=128))
            bt = cp.tile([BQ, H], DT.float32, tag="bhn")
            nc.sync.dma_start(bt[:], bhn[:])
            idt = cp.tile([128, 128], DT.float32, tag="id")
            nc.sync.dma_start(idt[:], ident[:])
            h_prev = hp.tile([BQ, H], DT.float32, tag="h")
            nc.sync.dma_start(h_prev[:], h0[:])
            ht_prev = tp.tile([128, 8, BQ], DT.float32r, tag="ht")
            nc.sync.dma_start(ht_prev[:], ht0[:])
            for t in range(T):
                g = gp.tile([BQ, G], DT.float32, tag="gi")
                nc.sync.dma_start(g[:], gi[t])
                # per-bank PSUM tiles: Tile tracks deps per tile, so a
                # single 6-bank tile would serialize each step end-to-end
                accs = [ps.tile([BQ, 512], DT.float32, tag=f"acc{b}",
                                name=f"acc{b}_{t}") for b in range(6)]
                # bank order: half-0's gate banks (r0,z0,n0) first so its
                # elementwise overlaps the remaining matmuls
                for b in (0, 2, 4, 1, 3, 5):
                    for k in range(8):
                        nc.tensor.matmul(accs[b][:], ht_prev[:, k, :],
                                         wt[:, k, 512 * b:512 * (b + 1)],
                                         start=(k == 0), stop=(k == 7))
                h_new = hp.tile([BQ, H], DT.float32, tag="h")
                ht_new = tp.tile([128, 8, BQ], DT.float32r, tag="ht")
                for j in (0, 1):
                    hs = slice(512 * j, 512 * (j + 1))        # hidden slice
                    rs = slice(512 * j, 512 * (j + 1))        # r cols
                    zs = slice(H + 512 * j, H + 512 * (j + 1))
                    ns = slice(2 * H + 512 * j, 2 * H + 512 * (j + 1))
                    pre = ep.tile([BQ, 1024], DT.float32, tag=f"pre{j}")
                    nc.vector.tensor_add(pre[:, 0:512], accs[j][:], g[:, rs])
                    nc.vector.tensor_add(pre[:, 512:1024], accs[2 + j][:], g[:, zs])
                    rz = ep.tile([BQ, 1024], DT.float32, tag=f"rz{j}")
                    nc.scalar.activation(rz[:], pre[:], AF.Sigmoid)
                    hnb = ep.tile([BQ, 512], DT.float32, tag=f"hn{j}")
                    nc.vector.tensor_add(hnb[:], accs[4 + j][:], bt[:, hs])
                    rhn = ep.tile([BQ, 512], DT.float32, tag=f"rhn{j}")
                    nc.vector.tensor_mul(rhn[:], rz[:, 0:512], hnb[:])
                    npre = ep.tile([BQ, 512], DT.float32, tag=f"np{j}")
                    nc.vector.tensor_add(npre[:], g[:, ns], rhn[:])
                    nt = ep.tile([BQ, 512], DT.float32, tag=f"n{j}")
                    nc.scalar.activation(nt[:], npre[:], AF.Tanh)
                    d = ep.tile([BQ, 512], DT.float32, tag=f"d{j}")
                    nc.vector.tensor_sub(d[:], h_prev[:, hs], nt[:])
                    e = ep.tile([BQ, 512], DT.float32, tag=f"e{j}")
                    nc.vector.tensor_mul(e[:], rz[:, 512:1024], d[:])
                    nc.vector.tensor_add(h_new[:, hs], nt[:], e[:])
                    for c in range(4):
                        kk = 4 * j + c
                        tr = pst.tile([128, BQ], DT.float32, tag="tr")
                        nc.tensor.transpose(tr[:], h_new[:, 128 * kk:128 * (kk + 1)],
                                            idt[0:BQ, 0:BQ])
                        nc.vector.tensor_copy(ht_new[:, kk, :], tr[:])
                if emit_ht:
                    nc.sync.dma_start(HT[t], ht_new[:])
                if emit_h:
                    nc.sync.dma_start(HO[t], h_new[:])
                h_prev, ht_prev = h_new, ht_new
            nc.sync.dma_start(hn[:], h_prev[:])
            nc.sync.dma_start(htn[:], ht_prev[:])
    _split_multiwaits(nc)
    return nc


# ----------------------------------------------------------------- jax runner

_FNS = {}
_MESH = [None]


def _fetch(garr):
    """Fetch a sharded global array pulling the 8 per-device shards in
    parallel (the tunnel serializes single-stream fetches)."""
    from concurrent.futures import ThreadPoolExecutor
    shards = list(garr.addressable_shards)
    shards.sort(key=lambda sh: (sh.index[0].start or 0) if sh.index else 0)
    with ThreadPoolExecutor(len(shards)) as ex:
        parts = list(ex.map(lambda sh: np.asarray(sh.data), shards))
    return np.concatenate(parts, axis=0)


def _to_dev(arr):
    """Place a global [8*d0, ...] array on the 8-core mesh once."""
    import jax
    from jax.sharding import NamedSharding, PartitionSpec
    return jax.device_put(arr, NamedSharding(_MESH[0], PartitionSpec("core")))


def _make_runner(key, nc):
    """Sharded executor over 8 cores keeping arrays on device (modeled on
    bass2jax.run_bass_via_pjrt, minus the host round-trips)."""
    import jax
    import jax.numpy as jnp
    from jax.sharding import Mesh, PartitionSpec
    from jax.experimental.shard_map import shard_map
    from concourse import bass2jax as b2j

    b2j.install_neuronx_cc_hook()
    partition_name = (nc.partition_id_tensor.name
                      if nc.partition_id_tensor else None)
    in_names, out_names, out_avals = [], [], []
    for alloc in nc.m.functions[0].allocations:
        if not isinstance(alloc, mybir.MemoryLocationSet):
            continue
        name = alloc.memorylocations[0].name
        if alloc.kind == "ExternalInput":
            if name != partition_name:
                in_names.append(name)
        elif alloc.kind == "ExternalOutput":
            out_names.append(name)
            out_avals.append(jax.core.ShapedArray(
                tuple(alloc.tensor_shape), mybir.dt.np(alloc.dtype)))
    n_params = len(in_names)
    all_in = in_names + out_names + ([partition_name] if partition_name else [])

    def _body(*args):
        operands = list(args)
        if partition_name is not None:
            operands.append(b2j.partition_id_tensor())
        return tuple(b2j._bass_exec_p.bind(
            *operands, out_avals=tuple(out_avals), in_names=tuple(all_in),
            out_names=tuple(out_names), lowering_input_output_aliases=(),
            sim_require_finite=False, sim_require_nnan=False, nc=nc))

    devices = jax.devices()[:NCORES]
    mesh = Mesh(np.asarray(devices), ("core",))
    _MESH[0] = mesh
    n_out = len(out_names)
    sharded = jax.jit(
        shard_map(_body, mesh=mesh,
                  in_specs=(PartitionSpec("core"),) * (n_params + n_out),
                  out_specs=(PartitionSpec("core"),) * n_out,
                  check_rep=False),
        donate_argnums=tuple(range(n_params, n_params + n_out)),
        keep_unused=True)

    def run(in_globals):
        """in_globals: dict name -> global array [8*d0, ...] (jax or np)."""
        import jax.numpy as jnp
        args = [in_globals[n] for n in in_names]
        zeros = [jnp.zeros((NCORES * a.shape[0], *a.shape[1:]), a.dtype)
                 for a in out_avals]
        outs = sharded(*args, *zeros)
        return dict(zip(out_names, outs))

    _FNS[key] = (run, in_names, out_names)
    return run


def _runner(key, builder):
    if key not in _FNS:
        _make_runner(key, builder())
    return _FNS[key][0]


# ----------------------------------------------------------------- host side


def _gather(per_core):
    """list of 8 per-core np arrays -> one global array (axis0 concat)."""
    return np.ascontiguousarray(np.concatenate(per_core, axis=0))


VERBOSE = os.environ.get("GRU_VERBOSE", "0") == "1"


def _tick(label, t0=[None]):
    import time
    now = time.time()
    if VERBOSE and t0[0] is not None:
        print(f"  [kernel] {label}: {now - t0[0]:.2f}s", flush=True)
    t0[0] = now


def kernel(x,
           w_ih0f, w_hh0f, b_ih0f, b_hh0f,
           w_ih0b, w_hh0b, b_ih0b, b_hh0b,
           w_ih1f, w_hh1f, b_ih1f, b_hh1f,
           w_ih1b, w_hh1b, b_ih1b, b_hh1b):
    import jax.numpy as jnp

    x = np.asarray(x, np.float32)
    f32 = lambda a: np.asarray(a, np.float32)
    # backward cell consumes x_t[:, ::-1]  ==  x_t @ flip(w_ih0b, axis=1)^T
    wih0 = {"f": f32(w_ih0f), "b": f32(w_ih0b)[:, ::-1]}
    whh0 = {"f": f32(w_hh0f), "b": f32(w_hh0b)}
    wih1 = {"f": f32(w_ih1f), "b": f32(w_ih1b)}
    whh1 = {"f": f32(w_hh1f), "b": f32(w_hh1b)}
    bias0 = {c: np.concatenate([
        (f32(bi)[:H] + f32(bh)[:H]),
        (f32(bi)[H:2 * H] + f32(bh)[H:2 * H]),
        f32(bi)[2 * H:]])
        for c, (bi, bh) in {"f": (b_ih0f, b_hh0f), "b": (b_ih0b, b_hh0b)}.items()}
    bias1 = {c: np.concatenate([
        (f32(bi)[:H] + f32(bh)[:H]),
        (f32(bi)[H:2 * H] + f32(bh)[H:2 * H]),
        f32(bi)[2 * H:]])
        for c, (bi, bh) in {"f": (b_ih1f, b_hh1f), "b": (b_ih1b, b_hh1b)}.items()}
    bhn0 = {c: f32(bh)[2 * H:] for c, bh in {"f": b_hh0f, "b": b_hh0b}.items()}
    bhn1 = {c: f32(bh)[2 * H:] for c, bh in {"f": b_hh1f, "b": b_hh1b}.items()}

    cores = [("f", q) for q in range(4)] + [("b", q) for q in range(4)]

    # --- phase A inputs: XT [IN, S*BQ] per core, W [IN, G], bias rep [128, G]
    xt_pc, wA_pc, biasA_pc = [], [], []
    for c, q in cores:
        xq = x[BQ * q:BQ * (q + 1)]                     # [BQ, S, IN]
        xt_pc.append(np.ascontiguousarray(
            xq.transpose(2, 1, 0).reshape(IN, S * BQ)))  # [(IN), (t b)]
        wA_pc.append(np.ascontiguousarray(wih0[c].T))    # [IN, G]
        biasA_pc.append(np.broadcast_to(bias0[c], (128, G)).copy())
    _tick("prep")
    runA = _runner("gemmA", lambda: build_gemm(IN // 128, False))
    outA = runA({"xt": _gather(xt_pc), "w": _gather(wA_pc),
                 "bias": _gather(biasA_pc)})
    _tick("gemmA")
    gi0 = outA["gi"]                                     # [8*8192, G] on device

    # --- scan 0
    runS0 = _runner("scan_ht", lambda: build_scan(True, False))
    whh0_pc = _to_dev(_gather([np.ascontiguousarray(whh0[c].T) for c, q in cores]))
    bhn0_pc = _to_dev(_gather([np.broadcast_to(bhn0[c], (BQ, H)).copy()
                               for c, q in cores]))
    ident = _to_dev(_gather([np.eye(128, dtype=np.float32)] * NCORES))
    h = jnp.zeros((NCORES * BQ, H), np.float32)
    ht = jnp.zeros((NCORES * 128, 8, BQ), np.float32)
    gi0_v = gi0.reshape(NCORES, S, BQ, G)
    hts = []
    for ch in range(S // TCHUNK):
        gi_sl = gi0_v[:, TCHUNK * ch:TCHUNK * (ch + 1)].reshape(
            NCORES * TCHUNK, BQ, G)
        o = runS0({"whh": whh0_pc, "gi": gi_sl, "bhn": bhn0_pc, "h0": h,
                   "ht0": ht, "ident": ident})
        h, ht = o["hn"], o["htn"]
        hts.append(o["HT"])

    _tick("scan0")
    # --- phase C: gi1 from scan-0 states
    runC = _runner("gemmC", lambda: build_gemm(H // 128, True))
    inC = {f"xt{i}": hts[i] for i in range(len(hts))}
    inC["w"] = _gather([np.ascontiguousarray(wih1[c].T) for c, q in cores])
    inC["bias"] = _gather([np.broadcast_to(bias1[c], (128, G)).copy()
                           for c, q in cores])
    gi1 = runC(inC)["gi"]

    _tick("gemmC")
    # --- scan 1
    runS1 = _runner("scan_h", lambda: build_scan(False, True))
    whh1_pc = _to_dev(_gather([np.ascontiguousarray(whh1[c].T) for c, q in cores]))
    bhn1_pc = _to_dev(_gather([np.broadcast_to(bhn1[c], (BQ, H)).copy()
                               for c, q in cores]))
    h = jnp.zeros((NCORES * BQ, H), np.float32)
    ht = jnp.zeros((NCORES * 128, 8, BQ), np.float32)
    gi1_v = gi1.reshape(NCORES, S, BQ, G)
    hos = []
    for ch in range(S // TCHUNK):
        gi_sl = gi1_v[:, TCHUNK * ch:TCHUNK * (ch + 1)].reshape(
            NCORES * TCHUNK, BQ, G)
        o = runS1({"whh": whh1_pc, "gi": gi_sl, "bhn": bhn1_pc, "h0": h,
                   "ht0": ht, "ident": ident})
        h, ht = o["hn"], o["htn"]
        hos.append(o["HO"])

    _tick("scan1")
    # --- assemble [B, S, 2H]
    import jax.numpy as _jnp
    h1_dev = [_jnp.transpose(o.reshape(NCORES, TCHUNK, BQ, H), (0, 2, 1, 3))
              for o in hos]                              # [core, b, t, H] chunks
    h1 = np.concatenate([np.asarray(o) for o in h1_dev], axis=2) \
        if len(h1_dev) > 1 else np.asarray(h1_dev[0])    # [core, b, S, H]
    out = np.empty((B, S, 2 * H), np.float32)
    for i, (c, q) in enumerate(cores):
        col = slice(0, H) if c == "f" else slice(H, 2 * H)
        out[BQ * q:BQ * (q + 1), :, col] = h1[i]
    _tick("assemble")
    return out, np.ascontiguousarray(out[:, -1, :])


# revision 12
# speedup vs baseline: 1755.8731x; 1.1360x over previous
"""Bidirectional 2-layer GRU encoder on 8 Trainium2 NeuronCores.

B=64, S=512, IN=512, H=1024. The reference's "backward" direction is a
feature-axis flip of x_t (not time reversal), so all 4 GRU cells scan forward
in time. Sharding: 8 cores = 2 cells (f/b chain) x 4 batch quarters (16 rows
each); each core runs its chain completely locally (no cross-core comm):

  phase A  : gi0 = Xq @ w_ih0^T + bias           (big GEMM, per core)
  scan 0   : layer-0 GRU scan, 512 steps          (emits transposed h states)
  phase C  : gi1 = H0q @ w_ih1^T + bias           (big GEMM from scan-0 states)
  scan 1   : layer-1 GRU scan, 512 steps          (emits h1 sequence)

Recurrent matmuls run with the batch-transposed state as the PE stationary
operand and w_hh^T streaming (fp32r: 1 cycle/row at N=512). Biases are
pre-folded into gi (b_ih for all gates + b_hh for r,z); b_hh_n is added on
the h-side inside the scan, matching PyTorch GRU gate math exactly.
"""

import os
import sys
import numpy as np

import concourse.bass as bass
import concourse.mybir as mybir
from concourse.tile import TileContext
from concourse.vector_clock import ScopedClock

B, S, IN, H = 64, 512, 512, 1024
G = 3 * H            # 3072 gate columns, order [r | z | n]
NCORES = 8
BQ = B // 4          # 16 batch rows per core
TCHUNK = 512         # scan steps per launch
AF = mybir.ActivationFunctionType
DT = mybir.dt

# ----------------------------------------------------------------- walrus fixes


def _patched_drain_and_barrier(self, tick_clock, wait_clock):
    nc = self.nc
    probe = nc.sync.nop(nofuse=True)
    wait_clock.add_sem_waits(probe.ins, ScopedClock({None: tick_clock.global_clock}))
    si = probe.ins.sync_info
    waits = list(si.on_wait) if si is not None else []
    probe.ins.sync_info = mybir.SyncInfo(on_wait=waits[:1], on_update=[])
    for w in waits[1:]:
        n2 = nc.sync.nop(nofuse=True)
        n2.ins.sync_info = mybir.SyncInfo(on_wait=[w], on_update=[])
    nc.sync.drain()
    nc.all_engine_barrier()
    popped = nc._tile_sem_poison_stack.pop()
    assert popped is self._sem_poison
    nc.clear_and_free_semaphores(list(self.sems.allocated().values()))
    nc.all_engine_barrier()


TileContext._drain_and_barrier = _patched_drain_and_barrier


def _split_multiwaits(nc):
    """This container's walrus accepts at most one sync-wait per instruction;
    hoist extras onto same-engine NOPs (sequencers are strict FIFO)."""
    for f in nc.m.functions:
        for bb in f.blocks:
            insts = bb.instructions
            i = 0
            while i < len(insts):
                inst = insts[i]
                si = inst.sync_info
                if si is not None and len(si.on_wait) > 1:
                    waits = list(si.on_wait)
                    for j, w in enumerate(waits[:-1]):
                        nop = mybir.InstNoOp(
                            name=f"{inst.name}.wsplit{j}", engine=inst.engine,
                            sync_info=mybir.SyncInfo(on_wait=[w], on_update=[]),
                            bass_nofuse=True)
                        insts.insert(i, nop)
                        i += 1
                    inst.sync_info = mybir.SyncInfo(
                        on_wait=[waits[-1]], on_update=list(si.on_update))
                i += 1


# ----------------------------------------------------------------- kernel builders


def build_gemm(k_chunks, from_ht):
    """gi[MR, G] = XT^T @ W + bias.  XT is the pre-transposed left operand
    [128*k_chunks, MR] (from_ht=False, one DRAM tensor) or four scan-chunk
    state dumps [TCHUNK, 128, 8, 16] (from_ht=True). W is [128*k_chunks, G].
    bias is replicated [128, G]. Row m of gi is (t, b) = (m // 16, m % 16)."""
    MR = S * BQ  # 8192
    nc = bass.Bass(trn_type="TRN2", num_devices=NCORES)
    if from_ht:
        xts = [nc.dram_tensor(f"xt{i}", [TCHUNK, 128, 8, BQ], DT.float32r,
                              kind="ExternalInput") for i in range(S // TCHUNK)]
    else:
        xt = nc.dram_tensor("xt", [128 * k_chunks, MR], DT.float32r,
                            kind="ExternalInput")
    w = nc.dram_tensor("w", [128 * k_chunks, G], DT.float32r, kind="ExternalInput")
    bias = nc.dram_tensor("bias", [128, G], DT.float32, kind="ExternalInput")
    gi = nc.dram_tensor("gi", [MR, G], DT.float32, kind="ExternalOutput")
    n_m = MR // 128  # 64 m-chunks; m-chunk = 8 consecutive steps x 16 batch
    with TileContext(nc) as tc:
        with tc.tile_pool(name="wp", bufs=1) as wp, \
             tc.tile_pool(name="xp", bufs=3) as xp, \
             tc.tile_pool(name="op", bufs=2) as op, \
             tc.tile_pool(name="bp", bufs=1) as bp, \
             tc.tile_pool(name="ps", bufs=4, space="PSUM") as ps:
            wt = wp.tile([128, k_chunks, G], DT.float32r)
            nc.sync.dma_start(wt[:], w.rearrange("(k p) g -> p k g", p=128))
            bt = bp.tile([128, G], DT.float32)
            nc.sync.dma_start(bt[:], bias[:])
            for m in range(n_m):
                if from_ht:
                    x4 = xp.tile([128, k_chunks, 8, BQ], DT.float32r, tag="x")
                    c, r = divmod(m, n_m // len(xts))
                    nc.sync.dma_start(
                        x4[:], xts[c][8 * r:8 * r + 8].rearrange("t p k b -> p k t b"))
                    x = x4.rearrange("p k t b -> p k (t b)")
                else:
                    x = xp.tile([128, k_chunks, 128], DT.float32r, tag="x")
                    nc.sync.dma_start(x[:], xt.rearrange("(k p) m -> p k m", p=128)[
                        :, :, 128 * m:128 * (m + 1)])
                ot = op.tile([128, G], DT.float32, tag="o")
                for b in range(G // 512):
                    acc = ps.tile([128, 512], DT.float32, tag="acc")
                    for k in range(k_chunks):
                        nc.tensor.matmul(acc[:], x[:, k, :],
                                         wt[:, k, 512 * b:512 * (b + 1)],
                                         start=(k == 0), stop=(k == k_chunks - 1))
                    nc.vector.tensor_add(ot[:, 512 * b:512 * (b + 1)], acc[:],
                                         bt[:, 512 * b:512 * (b + 1)])
                nc.sync.dma_start(gi[128 * m:128 * (m + 1), :], ot[:])
    _split_multiwaits(nc)
    return nc


def build_scan(emit_ht, emit_h):
    """TCHUNK GRU steps for one cell on BQ batch rows.
    State: h [BQ, H] and its transpose ht [128, 8, BQ] (fp32r, PE stationary)."""
    T = TCHUNK
    nc = bass.Bass(trn_type="TRN2", num_devices=NCORES)
    whh = nc.dram_tensor("whh", [H, G], DT.float32r, kind="ExternalInput")
    gi = nc.dram_tensor("gi", [T, BQ, G], DT.float32, kind="ExternalInput")
    bhn = nc.dram_tensor("bhn", [BQ, H], DT.float32, kind="ExternalInput")
    h0 = nc.dram_tensor("h0", [BQ, H], DT.float32, kind="ExternalInput")
    ht0 = nc.dram_tensor("ht0", [128, 8, BQ], DT.float32r, kind="ExternalInput")
    ident = nc.dram_tensor("ident", [128, 128], DT.float32, kind="ExternalInput")
    hn = nc.dram_tensor("hn", [BQ, H], DT.float32, kind="ExternalOutput")
    htn = nc.dram_tensor("htn", [128, 8, BQ], DT.float32r, kind="ExternalOutput")
    if emit_ht:
        HT = nc.dram_tensor("HT", [T, 128, 8, BQ], DT.float32r,
                            kind="ExternalOutput")
    if emit_h:
        HO = nc.dram_tensor("HO", [T, BQ, H], DT.float32, kind="ExternalOutput")
    with TileContext(nc) as tc:
        with tc.tile_pool(name="wp", bufs=1) as wp, \
             tc.tile_pool(name="cp", bufs=1) as cp, \
             tc.tile_pool(name="gp", bufs=2) as gp, \
             tc.tile_pool(name="hp", bufs=2) as hp, \
             tc.tile_pool(name="tp", bufs=2) as tp, \
             tc.tile_pool(name="ep", bufs=1) as ep, \
             tc.tile_pool(name="ps", bufs=1, space="PSUM") as ps, \
             tc.tile_pool(name="pst", bufs=2, space="PSUM") as pst:
            wt = wp.tile([128, 8, G], DT.float32r)
            nc.sync.dma_start(wt[:], whh.rearrange("(k p) g -> p k g", p=128))
            bt = cp.tile([BQ, H], DT.float32, tag="bhn")
            nc.sync.dma_start(bt[:], bhn[:])
            idt = cp.tile([128, 128], DT.float32, tag="id")
            nc.sync.dma_start(idt[:], ident[:])
            h_prev = hp.tile([BQ, H], DT.float32, tag="h")
            nc.sync.dma_start(h_prev[:], h0[:])
            ht_prev = tp.tile([128, 8, BQ], DT.float32r, tag="ht")
            nc.sync.dma_start(ht_prev[:], ht0[:])
            for t in range(T):
                g = gp.tile([BQ, G], DT.float32, tag="gi")
                nc.sync.dma_start(g[:], gi[t])
                # per-bank PSUM tiles: Tile tracks deps per tile, so a
                # single 6-bank tile would serialize each step end-to-end
                accs = [ps.tile([BQ, 512], DT.float32, tag=f"acc{b}",
                                name=f"acc{b}_{t}") for b in range(6)]
                # bank order: z-banks (2,3) last — the z-gate enters the
                # update chain at the very end (e = z*(h-n)), so everything
                # else overlaps the remaining matmuls
                for b in (0, 4, 2, 1, 5, 3):
                    for k in range(8):
                        nc.tensor.matmul(accs[b][:], ht_prev[:, k, :],
                                         wt[:, k, 512 * b:512 * (b + 1)],
                                         start=(k == 0), stop=(k == 7))
                h_new = hp.tile([BQ, H], DT.float32, tag="h")
                ht_new = tp.tile([128, 8, BQ], DT.float32r, tag="ht")
                for j in (0, 1):
                    hs = slice(512 * j, 512 * (j + 1))        # hidden slice
                    rs = slice(512 * j, 512 * (j + 1))        # r cols
                    zs = slice(H + 512 * j, H + 512 * (j + 1))
                    ns = slice(2 * H + 512 * j, 2 * H + 512 * (j + 1))
                    # r/sigmoid split from z/sigmoid: the r-gate feeds the
                    # long n-path, the z-gate is only needed for the final
                    # e = z*(h-n), so it can wait for the late z-bank
                    prer = ep.tile([BQ, 512], DT.float32, tag=f"prer{j}")
                    nc.vector.tensor_add(prer[:], accs[j][:], g[:, rs])
                    r = ep.tile([BQ, 512], DT.float32, tag=f"r{j}")
                    nc.scalar.activation(r[:], prer[:], AF.Sigmoid)
                    hnb = ep.tile([BQ, 512], DT.float32, tag=f"hn{j}")
                    nc.vector.tensor_add(hnb[:], accs[4 + j][:], bt[:, hs])
                    rhn = ep.tile([BQ, 512], DT.float32, tag=f"rhn{j}")
                    nc.vector.tensor_mul(rhn[:], r[:], hnb[:])
                    npre = ep.tile([BQ, 512], DT.float32, tag=f"np{j}")
                    nc.vector.tensor_add(npre[:], g[:, ns], rhn[:])
                    nt = ep.tile([BQ, 512], DT.float32, tag=f"n{j}")
                    nc.scalar.activation(nt[:], npre[:], AF.Tanh)
                    d = ep.tile([BQ, 512], DT.float32, tag=f"d{j}")
                    nc.vector.tensor_sub(d[:], h_prev[:, hs], nt[:])
                    prez = ep.tile([BQ, 512], DT.float32, tag=f"prez{j}")
                    nc.vector.tensor_add(prez[:], accs[2 + j][:], g[:, zs])
                    z = ep.tile([BQ, 512], DT.float32, tag=f"z{j}")
                    nc.scalar.activation(z[:], prez[:], AF.Sigmoid)
                    e = ep.tile([BQ, 512], DT.float32, tag=f"e{j}")
                    nc.vector.tensor_mul(e[:], z[:], d[:])
                    nc.vector.tensor_add(h_new[:, hs], nt[:], e[:])
                    for c in range(4):
                        kk = 4 * j + c
                        tr = pst.tile([128, BQ], DT.float32, tag="tr")
                        nc.tensor.transpose(tr[:], h_new[:, 128 * kk:128 * (kk + 1)],
                                            idt[0:BQ, 0:BQ])
                        nc.vector.tensor_copy(ht_new[:, kk, :], tr[:])
                if emit_ht:
                    nc.sync.dma_start(HT[t], ht_new[:])
                if emit_h:
                    nc.sync.dma_start(HO[t], h_new[:])
                h_prev, ht_prev = h_new, ht_new
            nc.sync.dma_start(hn[:], h_prev[:])
            nc.sync.dma_start(htn[:], ht_prev[:])
    _split_multiwaits(nc)
    return nc


# ----------------------------------------------------------------- jax runner

_FNS = {}
_MESH = [None]


def _fetch(garr):
    """Fetch a sharded global array pulling the 8 per-device shards in
    parallel (the tunnel serializes single-stream fetches)."""
    from concurrent.futures import ThreadPoolExecutor
    shards = list(garr.addressable_shards)
    shards.sort(key=lambda sh: (sh.index[0].start or 0) if sh.index else 0)
    with ThreadPoolExecutor(len(shards)) as ex:
        parts = list(ex.map(lambda sh: np.asarray(sh.data), shards))
    return np.concatenate(parts, axis=0)


def _to_dev(arr):
    """Place a global [8*d0, ...] array on the 8-core mesh once."""
    import jax
    from jax.sharding import NamedSharding, PartitionSpec
    return jax.device_put(arr, NamedSharding(_MESH[0], PartitionSpec("core")))


def _make_runner(key, nc):
    """Sharded executor over 8 cores keeping arrays on device (modeled on
    bass2jax.run_bass_via_pjrt, minus the host round-trips)."""
    import jax
    import jax.numpy as jnp
    from jax.sharding import Mesh, PartitionSpec
    from jax.experimental.shard_map import shard_map
    from concourse import bass2jax as b2j

    b2j.install_neuronx_cc_hook()
    partition_name = (nc.partition_id_tensor.name
                      if nc.partition_id_tensor else None)
    in_names, out_names, out_avals = [], [], []
    for alloc in nc.m.functions[0].allocations:
        if not isinstance(alloc, mybir.MemoryLocationSet):
            continue
        name = alloc.memorylocations[0].name
        if alloc.kind == "ExternalInput":
            if name != partition_name:
                in_names.append(name)
        elif alloc.kind == "ExternalOutput":
            out_names.append(name)
            out_avals.append(jax.core.ShapedArray(
                tuple(alloc.tensor_shape), mybir.dt.np(alloc.dtype)))
    n_params = len(in_names)
    all_in = in_names + out_names + ([partition_name] if partition_name else [])

    def _body(*args):
        operands = list(args)
        if partition_name is not None:
            operands.append(b2j.partition_id_tensor())
        return tuple(b2j._bass_exec_p.bind(
            *operands, out_avals=tuple(out_avals), in_names=tuple(all_in),
            out_names=tuple(out_names), lowering_input_output_aliases=(),
            sim_require_finite=False, sim_require_nnan=False, nc=nc))

    devices = jax.devices()[:NCORES]
    mesh = Mesh(np.asarray(devices), ("core",))
    _MESH[0] = mesh
    n_out = len(out_names)
    sharded = jax.jit(
        shard_map(_body, mesh=mesh,
                  in_specs=(PartitionSpec("core"),) * (n_params + n_out),
                  out_specs=(PartitionSpec("core"),) * n_out,
                  check_rep=False),
        donate_argnums=tuple(range(n_params, n_params + n_out)),
        keep_unused=True)

    def run(in_globals):
        """in_globals: dict name -> global array [8*d0, ...] (jax or np)."""
        import jax.numpy as jnp
        args = [in_globals[n] for n in in_names]
        zeros = [jnp.zeros((NCORES * a.shape[0], *a.shape[1:]), a.dtype)
                 for a in out_avals]
        outs = sharded(*args, *zeros)
        return dict(zip(out_names, outs))

    _FNS[key] = (run, in_names, out_names)
    return run


def _runner(key, builder):
    if key not in _FNS:
        _make_runner(key, builder())
    return _FNS[key][0]


# ----------------------------------------------------------------- host side


def _gather(per_core):
    """list of 8 per-core np arrays -> one global array (axis0 concat)."""
    return np.ascontiguousarray(np.concatenate(per_core, axis=0))


VERBOSE = os.environ.get("GRU_VERBOSE", "0") == "1"


def _tick(label, t0=[None]):
    import time
    now = time.time()
    if VERBOSE and t0[0] is not None:
        print(f"  [kernel] {label}: {now - t0[0]:.2f}s", flush=True)
    t0[0] = now


def kernel(x,
           w_ih0f, w_hh0f, b_ih0f, b_hh0f,
           w_ih0b, w_hh0b, b_ih0b, b_hh0b,
           w_ih1f, w_hh1f, b_ih1f, b_hh1f,
           w_ih1b, w_hh1b, b_ih1b, b_hh1b):
    import jax.numpy as jnp

    x = np.asarray(x, np.float32)
    f32 = lambda a: np.asarray(a, np.float32)
    # backward cell consumes x_t[:, ::-1]  ==  x_t @ flip(w_ih0b, axis=1)^T
    wih0 = {"f": f32(w_ih0f), "b": f32(w_ih0b)[:, ::-1]}
    whh0 = {"f": f32(w_hh0f), "b": f32(w_hh0b)}
    wih1 = {"f": f32(w_ih1f), "b": f32(w_ih1b)}
    whh1 = {"f": f32(w_hh1f), "b": f32(w_hh1b)}
    bias0 = {c: np.concatenate([
        (f32(bi)[:H] + f32(bh)[:H]),
        (f32(bi)[H:2 * H] + f32(bh)[H:2 * H]),
        f32(bi)[2 * H:]])
        for c, (bi, bh) in {"f": (b_ih0f, b_hh0f), "b": (b_ih0b, b_hh0b)}.items()}
    bias1 = {c: np.concatenate([
        (f32(bi)[:H] + f32(bh)[:H]),
        (f32(bi)[H:2 * H] + f32(bh)[H:2 * H]),
        f32(bi)[2 * H:]])
        for c, (bi, bh) in {"f": (b_ih1f, b_hh1f), "b": (b_ih1b, b_hh1b)}.items()}
    bhn0 = {c: f32(bh)[2 * H:] for c, bh in {"f": b_hh0f, "b": b_hh0b}.items()}
    bhn1 = {c: f32(bh)[2 * H:] for c, bh in {"f": b_hh1f, "b": b_hh1b}.items()}

    cores = [("f", q) for q in range(4)] + [("b", q) for q in range(4)]

    # --- phase A inputs: XT [IN, S*BQ] per core, W [IN, G], bias rep [128, G]
    xt_pc, wA_pc, biasA_pc = [], [], []
    for c, q in cores:
        xq = x[BQ * q:BQ * (q + 1)]                     # [BQ, S, IN]
        xt_pc.append(np.ascontiguousarray(
            xq.transpose(2, 1, 0).reshape(IN, S * BQ)))  # [(IN), (t b)]
        wA_pc.append(np.ascontiguousarray(wih0[c].T))    # [IN, G]
        biasA_pc.append(np.broadcast_to(bias0[c], (128, G)).copy())
    _tick("prep")
    runA = _runner("gemmA", lambda: build_gemm(IN // 128, False))
    outA = runA({"xt": _gather(xt_pc), "w": _gather(wA_pc),
                 "bias": _gather(biasA_pc)})
    _tick("gemmA")
    gi0 = outA["gi"]                                     # [8*8192, G] on device

    # --- scan 0
    runS0 = _runner("scan_ht", lambda: build_scan(True, False))
    whh0_pc = _to_dev(_gather([np.ascontiguousarray(whh0[c].T) for c, q in cores]))
    bhn0_pc = _to_dev(_gather([np.broadcast_to(bhn0[c], (BQ, H)).copy()
                               for c, q in cores]))
    ident = _to_dev(_gather([np.eye(128, dtype=np.float32)] * NCORES))
    h = jnp.zeros((NCORES * BQ, H), np.float32)
    ht = jnp.zeros((NCORES * 128, 8, BQ), np.float32)
    gi0_v = gi0.reshape(NCORES, S, BQ, G)
    hts = []
    for ch in range(S // TCHUNK):
        gi_sl = gi0_v[:, TCHUNK * ch:TCHUNK * (ch + 1)].reshape(
            NCORES * TCHUNK, BQ, G)
        o = runS0({"whh": whh0_pc, "gi": gi_sl, "bhn": bhn0_pc, "h0": h,
                   "ht0": ht, "ident": ident})
        h, ht = o["hn"], o["htn"]
        hts.append(o["HT"])

    _tick("scan0")
    # --- phase C: gi1 from scan-0 states
    runC = _runner("gemmC", lambda: build_gemm(H // 128, True))
    inC = {f"xt{i}": hts[i] for i in range(len(hts))}
    inC["w"] = _gather([np.ascontiguousarray(wih1[c].T) for c, q in cores])
    inC["bias"] = _gather([np.broadcast_to(bias1[c], (128, G)).copy()
                           for c, q in cores])
    gi1 = runC(inC)["gi"]

    _tick("gemmC")
    # --- scan 1
    runS1 = _runner("scan_h", lambda: build_scan(False, True))
    whh1_pc = _to_dev(_gather([np.ascontiguousarray(whh1[c].T) for c, q in cores]))
    bhn1_pc = _to_dev(_gather([np.broadcast_to(bhn1[c], (BQ, H)).copy()
                               for c, q in cores]))
    h = jnp.zeros((NCORES * BQ, H), np.float32)
    ht = jnp.zeros((NCORES * 128, 8, BQ), np.float32)
    gi1_v = gi1.reshape(NCORES, S, BQ, G)
    hos = []
    for ch in range(S // TCHUNK):
        gi_sl = gi1_v[:, TCHUNK * ch:TCHUNK * (ch + 1)].reshape(
            NCORES * TCHUNK, BQ, G)
        o = runS1({"whh": whh1_pc, "gi": gi_sl, "bhn": bhn1_pc, "h0": h,
                   "ht0": ht, "ident": ident})
        h, ht = o["hn"], o["htn"]
        hos.append(o["HO"])

    _tick("scan1")
    # --- assemble [B, S, 2H]
    import jax.numpy as _jnp
    h1_dev = [_jnp.transpose(o.reshape(NCORES, TCHUNK, BQ, H), (0, 2, 1, 3))
              for o in hos]                              # [core, b, t, H] chunks
    h1 = np.concatenate([np.asarray(o) for o in h1_dev], axis=2) \
        if len(h1_dev) > 1 else np.asarray(h1_dev[0])    # [core, b, S, H]
    out = np.empty((B, S, 2 * H), np.float32)
    for i, (c, q) in enumerate(cores):
        col = slice(0, H) if c == "f" else slice(H, 2 * H)
        out[BQ * q:BQ * (q + 1), :, col] = h1[i]
    _tick("assemble")
    return out, np.ascontiguousarray(out[:, -1, :])
